# revision 29
# baseline (speedup 1.0000x reference)
"""PointAttention Trainium2 kernel — self-contained.

Math (per voxel b, M=32 points, d=64):
  dist[i,j,:] = |xyz[j]-xyz[i]|            (3)
  p   = relu(LN3(dist @ p_W1 + p_b1))
  pij = p @ p_W2 + p_b2                    (64)
  q/k = xyz @ Wq/Wk + b
  w0  = q_i - k_j + pij                    (scale dropped: LN-invariant)
  w   = relu(LN(w0)) @ w_W1 + w_b1
  w   = relu(LN(w)) @ w_W2 [+ w_b2 dropped: softmax-invariant]
  wij = softmax_j(w);  res[i] = wij[i,i,:] * (feat @ Wv + bv)
  out = concat([res, feat], -1)

Implementation: feature-major tiles [128 = 2 voxels x 64 feats, 1024 = (i,j)].
LN mean-subtraction folded into centered weights; variance via block-diagonal
ones matmuls on the PE (fp32r); relu/divide commuted so each LN costs one DVE
divide; q-k+pij assembled by 3 accumulating PE matmuls with broadcast rhs APs.
"""

import numpy as np
from contextlib import ExitStack

import concourse.bass as bass
import concourse.mybir as mybir
import concourse.tile as tile
from concourse import bass_utils

f32 = mybir.dt.float32
f32r = mybir.dt.float32r
AF = mybir.ActivationFunctionType
ALU = mybir.AluOpType
AX = mybir.AxisListType

B, M, DP, D = 1024, 32, 64, 64
N_CORES = 8
VB = B // N_CORES          # voxels per core
GV = 32                    # voxels per group
NG = VB // GV              # groups per core
NP = GV // 2               # pairs per group
EPS = 1e-5

MAX_WAITS = 1


def _split_sync_waits(nc, max_waits=MAX_WAITS):
    """Walrus in this container rejects >2 sem-waits per instruction; split
    extras onto same-engine InstDrain carriers placed just before."""
    n = 0
    for bb in nc.main_func.blocks:
        insts = bb.instructions
        out = []
        changed = False
        for ins in insts:
            si = ins.sync_info
            if si is not None and len(si.on_wait) > max_waits:
                waits = list(si.on_wait)
                extra, keep = waits[:-max_waits], waits[-max_waits:]
                for i in range(0, len(extra), max_waits):
                    n += 1
                    out.append(mybir.InstNoOp(
                        name=f"wsplit-{n}",
                        sync_info=mybir.SyncInfo(
                            on_wait=extra[i:i + max_waits], on_update=[]),
                        bass_nofuse=True, engine=ins.engine))
                ins.sync_info = mybir.SyncInfo(
                    on_wait=keep, on_update=list(si.on_update))
                changed = True
            out.append(ins)
        if changed:
            insts[:] = out


# row permutation taking (v,c)-ordered 96-rows to the (c,v) DMA layout
PERM96 = np.array([3 * (i % GV) + (i // GV) for i in range(96)])


def _wp2_all(pW2_c):
    out = np.zeros((96, NP * 128), np.float64)
    for p in range(NP):
        out[6 * p:6 * p + 3, 128 * p:128 * p + 64] = pW2_c
        out[6 * p + 3:6 * p + 6, 128 * p + 64:128 * p + 128] = pW2_c
    return out


def _wv_pk2(Wv):
    out = np.zeros((64, 256), np.float64)
    out[:, 0:64] = Wv
    out[:, 128 + 64:256] = Wv
    return out


def _wqk_all(Wc):
    # lhsT selecting pair p's two voxels from the (v,c) x i layout
    out = np.zeros((96, NP * 128), np.float64)
    for p in range(NP):
        for w in range(2):
            out[6 * p + 3 * w:6 * p + 3 * w + 3,
                128 * p + 64 * w:128 * p + 64 * w + 64] = Wc
    return out


def _blockdiag(w, n):
    """n copies of w on the diagonal: [n*k, n*m]."""
    k, m = w.shape
    out = np.zeros((n * k, n * m), np.float32)
    for i in range(n):
        out[i * k:(i + 1) * k, i * m:(i + 1) * m] = w
    return out


def prep_weights(inp):
    """Host-side weight preparation (float64 for exact centering)."""
    g = lambda k: np.asarray(inp[k], np.float64)
    Wq, bq = g("Wq"), g("bq")
    Wk, bk = g("Wk"), g("bk")
    Wv, bv = g("Wv"), g("bv")
    pW1, pb1 = g("p_W1"), g("p_b1")
    pW2, pb2 = g("p_W2"), g("p_b2")
    pg, pb = g("p_ln_g"), g("p_ln_b")
    g1, b1ln = g("w_ln1_g"), g("w_ln1_b")
    W1, b1 = g("w_W1"), g("w_b1")
    g2, b2ln = g("w_ln2_g"), g("w_ln2_b")
    W2 = g("w_W2")

    assert np.all(pb == 0) and np.all(b1ln == 0) and np.all(b2ln == 0), \
        "kernel assumes LN beta == 0 (as produced by setup_inputs)"

    cen = lambda w: w - w.mean(axis=1, keepdims=True)
    Wq_c, bq_c = cen(Wq), bq - bq.mean()
    Wk_c, bk_c = cen(Wk), bk - bk.mean()
    pW2_c, pb2_c = cen(pW2), pb2 - pb2.mean()
    pW1_c, pb1_c = cen(pW1), pb1 - pb1.mean()
    W1_c, b1_c = cen(W1), b1 - b1.mean()

    bias0 = bq_c - bk_c + pb2_c            # per-d constant in w0_c

    t2 = lambda v: np.tile(v, 2).astype(np.float32)
    f = lambda v: v.astype(np.float32)
    return {
        "wq_all": f(_wqk_all(Wq_c)[PERM96]),     # [96, 16*128]
        "wk_alln": f(_wqk_all(-Wk_c)[PERM96]),   # [96, 16*128]
        "wp2_all": f(_wp2_all(pW2_c)[PERM96]),   # [96, 16*128]
        "wp1_96": f(_blockdiag(pW1_c, GV)[PERM96][:, PERM96]),   # [96, 96]
        "ones3_96": f(_blockdiag(np.ones((3, 3)), GV)[PERM96][:, PERM96]),
        "w1c_128": f(_blockdiag(W1_c, 2)),       # [128, 128]
        "w2g_128": f(_blockdiag(W2, 2)),         # W2 plain (g2 applied in rw1)
        "ones64_128": f(_blockdiag(np.ones((64, 64)), 2)),
        "ident128": np.eye(128, dtype=np.float32),
        "wv_pk2": f(_wv_pk2(Wv)),                # [64, 256]
        "bias0": t2(bias0),                      # [128]
        "bias0g1": t2(bias0 * g1),               # [128]
        "g1v": t2(g1),                           # [128]
        "b1c": t2(b1_c),                         # [128]
        "g2v": t2(g2),                           # [128]
        "b1cg2": t2(b1_c * g2),                  # [128]
        "pb1c": np.tile(pb1_c, GV)[PERM96].astype(np.float32),        # [96]
        "gpv": np.tile(pg, GV)[PERM96].astype(np.float32),            # [96]
        "pb1cgp": (np.tile(pb1_c * pg, GV))[PERM96].astype(np.float32),  # [96]
        "bv2": t2(bv),                           # [128]
        "g2_is_one": bool(np.all(g2 == 1.0)),
    }


MM_WEIGHTS = ("wq_all", "wk_alln", "wp2_all", "wp1_96", "ones3_96",
              "w1c_128", "w2g_128", "ones64_128", "wv_pk2")

WEIGHT_SHAPES = {
    "wq_all": [96, NP * 128], "wk_alln": [96, NP * 128],
    "wp2_all": [96, NP * 128],
    "wp1_96": [96, 96], "ones3_96": [96, 96],
    "w1c_128": [128, 128], "w2g_128": [128, 128], "ones64_128": [128, 128],
    "ident128": [128, 128], "wv_pk2": [64, 256],
    "bias0": [128], "bias0g1": [128], "g1v": [128], "b1c": [128],
    "g2v": [128], "b1cg2": [128],
    "pb1c": [96], "gpv": [96], "pb1cgp": [96], "bv2": [128],
}


def build_module(nv=VB, g2_is_one=True, split_waits=True, reps=1):
    """Build the per-core Bass module for nv voxels (nv % 32 == 0)."""
    ng = nv // GV
    nc = bass.Bass()
    xyz_d = nc.declare_dram_parameter("xyz", [nv * M * 3], f32r, isOutput=False)
    feat_d = nc.declare_dram_parameter("feat", [nv * M * D], f32, isOutput=False)
    wt = {k: nc.declare_dram_parameter(k, s, f32r if k in MM_WEIGHTS else f32,
                                       isOutput=False)
          for k, s in WEIGHT_SHAPES.items()}
    out_d = nc.declare_dram_parameter("out", [nv * M * 2 * D], f32, isOutput=True)

    def vec_tile(pool, name, p):
        t = pool.tile([p, 1], f32, tag=name)
        nc.sync.dma_start(out=t, in_=wt[name][:].unsqueeze(1))
        return t

    with tile.TileContext(nc) as tc:
        with ExitStack() as ctx:
            wpool = ctx.enter_context(tc.tile_pool(name="wts", bufs=1))
            gpool = ctx.enter_context(tc.tile_pool(name="grp", bufs=2))
            ppool = ctx.enter_context(tc.tile_pool(name="pair", bufs=2))
            spool = ctx.enter_context(tc.tile_pool(name="small", bufs=3))
            ps_w0 = ctx.enter_context(tc.tile_pool(name="psw0", bufs=2, space="PSUM"))
            ps_s = ctx.enter_context(tc.tile_pool(name="pss", bufs=2, space="PSUM"))
            ps_p = ctx.enter_context(tc.tile_pool(name="psp", bufs=2, space="PSUM"))
            ps_t = ctx.enter_context(tc.tile_pool(name="pst", bufs=1, space="PSUM"))

            # ---- preload weights ----
            w_mm = {}
            for name in ("wq_all", "wk_alln", "wp2_all", "wp1_96", "ones3_96",
                         "w1c_128", "w2g_128", "ones64_128", "ident128",
                         "wv_pk2"):
                p, q = WEIGHT_SHAPES[name]
                t = wpool.tile([p, q], f32r if name in MM_WEIGHTS else f32,
                               tag=name)
                nc.sync.dma_start(out=t, in_=wt[name][:])
                w_mm[name] = t
            vt = {}
            for name in ("bias0", "bias0g1", "g1v", "b1c", "g2v", "b1cg2"):
                vt[name] = vec_tile(wpool, name, 128)
            for name in ("pb1c", "gpv", "pb1cgp"):
                vt[name] = vec_tile(wpool, name, 96)
            vt["bv2"] = vec_tile(wpool, "bv2", 128)
            eps128 = wpool.tile([128, 1], f32, tag="eps128")
            nc.vector.memset(eps128, EPS)
            eps96 = wpool.tile([96, 1], f32, tag="eps96")
            nc.vector.memset(eps96, EPS)
            zero1 = wpool.tile([128, 1], f32, tag="zero1")
            nc.vector.memset(zero1, 0.0)

            # ---- features passthrough: out[:, :, 64:128] = feat ----
            src = bass.AP(tensor=feat_d, offset=0,
                          ap=[[M * D, nv], [D, M], [1, D]])
            dst = bass.AP(tensor=out_d, offset=D,
                          ap=[[M * 2 * D, nv], [2 * D, M], [1, D]])
            nc.sync.dma_start(out=dst, in_=src)

            for _rep in range(reps):
              for g in range(ng):
                # ---- stage AB: per-group [96, *] tiles ----
                xyzT = gpool.tile([96, M], f32r, tag="xyzT")  # (v,c) x i
                for c in range(3):
                    nc.sync.dma_start(
                        out=bass.AP(tensor=xyzT.tensor,
                                    offset=xyzT.offset + c * GV * M,
                                    ap=[[M, GV], [1, M]]),
                        in_=bass.AP(tensor=xyz_d,
                                    offset=g * GV * M * 3 + c,
                                    ap=[[M * 3, GV], [3, M]]))

                # dist = x[c,j] - x[c,i]; adist = |dist|
                dist = gpool.tile([96, M * M], f32, tag="dist")
                in0 = bass.AP(tensor=xyzT.tensor, offset=xyzT.offset,
                              ap=[xyzT.ap[0], [0, M], [1, M]]).bitcast(f32)
                in1 = bass.AP(tensor=xyzT.tensor, offset=xyzT.offset,
                              ap=[xyzT.ap[0], [1, M], [0, M]]).bitcast(f32)
                dist3 = bass.AP(tensor=dist.tensor, offset=dist.offset,
                                ap=[dist.ap[0], [M, M], [1, M]])
                nc.vector.tensor_tensor(out=dist3, in0=in0, in1=in1,
                                        op=ALU.subtract)
                adist = gpool.tile([96, M * M], f32, tag="adist")
                nc.scalar.activation(out=adist.bitcast(f32r), in_=dist, func=AF.Abs)

                # p1c = blockdiag(p_W1_c) @ adist ; LN3 stats + relu
                sqB = gpool.tile([96, M * M], f32, tag="sqB")
                rp0 = gpool.tile([96, M * M], f32, tag="rp0")
                for h in range(2):
                    p1c_t = ps_s.tile([128, 512], f32, tag="S")
                    p1c = p1c_t[:96]
                    nc.tensor.matmul(out=p1c, lhsT=w_mm["wp1_96"].bitcast(f32r),
                                     rhs=adist[:, h * 512:(h + 1) * 512].bitcast(f32r),
                                     start=True, stop=True)
                    sl = slice(h * 512, (h + 1) * 512)
                    nc.scalar.activation(out=sqB[:, sl].bitcast(f32r), in_=p1c,
                                         func=AF.Square, bias=vt["pb1c"])
                    nc.scalar.activation(out=rp0[:, sl], in_=p1c, func=AF.Relu,
                                         scale=vt["gpv"], bias=vt["pb1cgp"])
                spB = gpool.tile([96, M * M], f32, tag="spB")
                for h in range(2):
                    Sp_t = ps_s.tile([128, 512], f32, tag="S")
                    Sp = Sp_t[:96]
                    nc.tensor.matmul(out=Sp, lhsT=w_mm["ones3_96"].bitcast(f32r),
                                     rhs=sqB[:, h * 512:(h + 1) * 512].bitcast(f32r),
                                     start=True, stop=True)
                    nc.scalar.activation(out=spB[:, h * 512:(h + 1) * 512],
                                         in_=Sp, func=AF.Sqrt,
                                         scale=1.0 / 3.0, bias=eps96)
                spBinv = gpool.tile([96, M * M], f32, tag="spBinv")
                nc.vector.reciprocal(spBinv, spB)
                relu_p0 = gpool.tile([96, M * M], f32, tag="relu_p0")
                nc.vector.tensor_tensor(out=relu_p0, in0=rp0, in1=spBinv,
                                        op=ALU.mult)
                relu_p = gpool.tile([96, M * M], f32, tag="relu_p")
                nc.gpsimd.tensor_scalar(out=relu_p.bitcast(f32r), in0=relu_p0,
                                        scalar1=0.0, scalar2=None, op0=ALU.add)

                # featT: [64, (v,i)] via PE transposes; v = Wv^T @ featT + bv
                featT = gpool.tile([64, GV * M], f32, tag="featT")
                for t in range(GV * M // 128):
                    ft = spool.tile([128, D], f32, tag="ft")
                    nc.sync.dma_start(
                        out=ft,
                        in_=bass.AP(tensor=feat_d,
                                    offset=(g * GV * M + t * 128) * D,
                                    ap=[[D, 128], [1, D]]))
                    ftp = ps_t.tile([64, 128], f32, tag="tr64")
                    nc.tensor.transpose(out=ftp, in_=ft,
                                        identity=w_mm["ident128"])
                    nc.vector.tensor_copy(featT[:, t * 128:(t + 1) * 128].bitcast(f32r), ftp)
                v_sb = gpool.tile([128, NP * M], f32, tag="v_sb")
                v_ps = ps_s.tile([128, 512], f32, tag="S")
                for w in range(2):
                    v_rhs = bass.AP(tensor=featT.tensor,
                                    offset=featT.offset + w * M,
                                    ap=[featT.ap[0], [2 * M, NP], [1, M]])
                    nc.tensor.matmul(
                        out=v_ps,
                        lhsT=w_mm["wv_pk2"][:, 128 * w:128 * (w + 1)],
                        rhs=v_rhs.bitcast(f32r),
                        start=(w == 0), stop=(w == 1))
                nc.scalar.activation(out=v_sb, in_=v_ps, func=AF.Identity,
                                     bias=vt["bv2"])

                # ---- pair loop ----
                for p in range(NP):
                    sq0 = ppool.tile([128, M * M], f32, tag="sq0")
                    r0 = ppool.tile([128, M * M], f32, tag="r0")
                    s1 = ppool.tile([128, M * M], f32, tag="s1")
                    d1 = ppool.tile([128, M * M], f32, tag="d1")
                    sq2 = ppool.tile([128, M * M], f32, tag="sq2")
                    rw1 = ppool.tile([128, M * M], f32, tag="rw1")
                    s2 = ppool.tile([128, M * M], f32, tag="s2")
                    t2t = ppool.tile([128, M * M], f32, tag="t2t")
                    Et = ppool.tile([128, M * M], f32, tag="Et")

                    for h in range(2):
                        sl = slice(h * 512, (h + 1) * 512)
                        w0 = ps_w0.tile([128, 512], f32, tag="w0")
                        nc.tensor.matmul(
                            out=w0,
                            lhsT=w_mm["wp2_all"][:, 128 * p:128 * (p + 1)].bitcast(f32r),
                            rhs=relu_p[:, sl].bitcast(f32r),
                            start=True, stop=False)
                        q_rhs = bass.AP(tensor=xyzT.tensor,
                                        offset=xyzT.offset + 16 * h,
                                        ap=[xyzT.ap[0], [1, 16], [0, M]])
                        nc.tensor.matmul(
                            out=w0,
                            lhsT=w_mm["wq_all"][:, 128 * p:128 * (p + 1)].bitcast(f32r),
                            rhs=q_rhs.bitcast(f32r),
                            start=False, stop=False)
                        k_rhs = bass.AP(tensor=xyzT.tensor,
                                        offset=xyzT.offset,
                                        ap=[xyzT.ap[0], [0, 16], [1, M]])
                        nc.tensor.matmul(
                            out=w0,
                            lhsT=w_mm["wk_alln"][:, 128 * p:128 * (p + 1)].bitcast(f32r),
                            rhs=k_rhs.bitcast(f32r),
                            start=False, stop=True)
                        # sq0 = (w0 + bias0)^2 ; r0 = relu(w0*g1 + bias0*g1)
                        nc.scalar.activation(out=sq0[:, sl].bitcast(f32r), in_=w0,
                                             func=AF.Square, bias=vt["bias0"])
                        nc.scalar.activation(out=r0[:, sl].bitcast(f32r), in_=w0,
                                             func=AF.Relu,
                                             scale=vt["g1v"], bias=vt["bias0g1"])
                    for h in range(2):
                        sl = slice(h * 512, (h + 1) * 512)
                        S1 = ps_s.tile([128, 512], f32, tag="S")
                        nc.tensor.matmul(out=S1,
                                         lhsT=w_mm["ones64_128"].bitcast(f32r),
                                         rhs=sq0[:, sl].bitcast(f32r),
                                         start=True, stop=True)
                        nc.scalar.activation(out=s1[:, sl], in_=S1,
                                             func=AF.Sqrt,
                                             scale=1.0 / 64.0, bias=eps128)
                    s1i = ppool.tile([128, M * M], f32, tag="s1i")
                    nc.vector.reciprocal(s1i, s1)
                    for h in range(2):
                        sl = slice(h * 512, (h + 1) * 512)
                        P1 = ps_p.tile([128, 512], f32, tag="P")
                        nc.tensor.matmul(out=P1, lhsT=w_mm["w1c_128"].bitcast(f32r),
                                         rhs=r0[:, sl].bitcast(f32r),
                                         start=True, stop=True)
                        nc.vector.tensor_tensor(out=d1[:, sl], in0=P1,
                                                in1=s1i[:, sl], op=ALU.mult)
                    # sq2 = (d1 + b1c)^2 ; rw1 = relu((d1 + b1c)*g2)
                    nc.scalar.activation(out=sq2.bitcast(f32r), in_=d1,
                                         func=AF.Square, bias=vt["b1c"])
                    if g2_is_one:
                        zb = bass.AP(tensor=zero1.tensor, offset=zero1.offset,
                                     ap=[zero1.ap[0], [0, M * M]])
                        nc.vector.scalar_tensor_tensor(
                            out=rw1.bitcast(f32r), in0=d1, scalar=vt["b1c"],
                            in1=zb, op0=ALU.add, op1=ALU.max)
                    else:
                        nc.scalar.activation(out=rw1.bitcast(f32r), in_=d1,
                                             func=AF.Relu,
                                             scale=vt["g2v"], bias=vt["b1cg2"])
                    for h in range(2):
                        sl = slice(h * 512, (h + 1) * 512)
                        S2 = ps_s.tile([128, 512], f32, tag="S")
                        nc.tensor.matmul(out=S2,
                                         lhsT=w_mm["ones64_128"].bitcast(f32r),
                                         rhs=sq2[:, sl].bitcast(f32r),
                                         start=True, stop=True)
                        nc.scalar.activation(out=s2[:, sl], in_=S2,
                                             func=AF.Sqrt,
                                             scale=1.0 / 64.0, bias=eps128)
                    s2i = ppool.tile([128, M * M], f32, tag="s2i")
                    nc.vector.reciprocal(s2i, s2)
                    for h in range(2):
                        sl = slice(h * 512, (h + 1) * 512)
                        P2 = ps_p.tile([128, 512], f32, tag="P")
                        nc.tensor.matmul(out=P2, lhsT=w_mm["w2g_128"].bitcast(f32r),
                                         rhs=rw1[:, sl].bitcast(f32r),
                                         start=True, stop=True)
                        nc.vector.tensor_tensor(out=t2t[:, sl], in0=P2,
                                                in1=s2i[:, sl], op=ALU.mult)
                    nc.scalar.activation(out=Et, in_=t2t, func=AF.Exp)

                    # softmax pieces
                    Ssum = spool.tile([128, M], f32, tag="Ssum")
                    E3 = bass.AP(tensor=Et.tensor, offset=Et.offset,
                                 ap=[Et.ap[0], [M, M], [1, M]])
                    nc.vector.tensor_reduce(out=Ssum, in_=E3, axis=AX.X, op=ALU.add)
                    Sinv = spool.tile([128, M], f32, tag="Sinv")
                    nc.vector.reciprocal(Sinv, Ssum)
                    gt = spool.tile([128, M], f32, tag="gt")
                    Ediag = bass.AP(tensor=Et.tensor, offset=Et.offset,
                                    ap=[Et.ap[0], [M + 1, M]])
                    nc.vector.tensor_tensor(out=gt, in0=Ediag, in1=Sinv,
                                            op=ALU.mult)
                    res = spool.tile([128, M], f32, tag="res")
                    nc.vector.tensor_tensor(
                        out=res, in0=gt,
                        in1=v_sb[:, p * M:(p + 1) * M], op=ALU.mult)
                    rtp = ps_t.tile([32, 128], f32, tag="rtp")
                    nc.tensor.transpose(out=rtp, in_=res,
                                        identity=w_mm["ident128"])
                    rts = spool.tile([32, 128], f32, tag="rts")
                    nc.vector.tensor_copy(rts, rtp)
                    nc.sync.dma_start(
                        out=bass.AP(tensor=out_d,
                                    offset=(g * GV + 2 * p) * M * 2 * D,
                                    ap=[[2 * D, M], [M * 2 * D, 2], [1, D]]),
                        in_=rts)

    if split_waits:
        _split_sync_waits(nc)
    return nc


_module_cache = {}


def _get_module(nv, g2_is_one):
    key = (nv, g2_is_one)
    if key not in _module_cache:
        _module_cache[key] = build_module(nv, g2_is_one)
    return _module_cache[key]


def kernel(**inputs):
    w = prep_weights(inputs)
    g2_is_one = w.pop("g2_is_one")
    nc = _get_module(VB, g2_is_one)

    xyz = np.ascontiguousarray(np.asarray(inputs["pv_xyz"], np.float32))
    feat = np.ascontiguousarray(np.asarray(inputs["features"], np.float32))
    in_maps = []
    for c in range(N_CORES):
        m = {k: v for k, v in w.items()}
        m["xyz"] = xyz[c * VB:(c + 1) * VB].reshape(-1)
        m["feat"] = feat[c * VB:(c + 1) * VB].reshape(-1)
        in_maps.append(m)
    res = bass_utils.run_bass_kernel_spmd(nc, in_maps,
                                          core_ids=list(range(N_CORES)))
    out = np.concatenate(
        [res.results[c]["out"].reshape(VB, M, 2 * D) for c in range(N_CORES)],
        axis=0)
    return out


# revision 30
# speedup vs baseline: 273.8278x; 273.8278x over previous
"""PointAttention Trainium2 kernel — self-contained.

Math (per voxel b, M=32 points, d=64):
  dist[i,j,:] = |xyz[j]-xyz[i]|            (3)
  p   = relu(LN3(dist @ p_W1 + p_b1))
  pij = p @ p_W2 + p_b2                    (64)
  q/k = xyz @ Wq/Wk + b
  w0  = q_i - k_j + pij                    (scale dropped: LN-invariant)
  w   = relu(LN(w0)) @ w_W1 + w_b1
  w   = relu(LN(w)) @ w_W2 [+ w_b2 dropped: softmax-invariant]
  wij = softmax_j(w);  res[i] = wij[i,i,:] * (feat @ Wv + bv)
  out = concat([res, feat], -1)

Implementation: feature-major tiles [128 = 2 voxels x 64 feats, 1024 = (i,j)].
LN mean-subtraction folded into centered weights; variance via block-diagonal
ones matmuls on the PE (fp32r); relu/divide commuted so each LN costs one DVE
divide; q-k+pij assembled by 3 accumulating PE matmuls with broadcast rhs APs.
"""

import numpy as np
from contextlib import ExitStack

import concourse.bass as bass
import concourse.mybir as mybir
import concourse.tile as tile
from concourse import bass_utils

f32 = mybir.dt.float32
f32r = mybir.dt.float32r
AF = mybir.ActivationFunctionType
ALU = mybir.AluOpType
AX = mybir.AxisListType

B, M, DP, D = 1024, 32, 64, 64
N_CORES = 8
VB = B // N_CORES          # voxels per core
GV = 32                    # voxels per group
NG = VB // GV              # groups per core
NP = GV // 2               # pairs per group
EPS = 1e-5

MAX_WAITS = 1


def _split_sync_waits(nc, max_waits=MAX_WAITS):
    """Walrus in this container rejects >2 sem-waits per instruction; split
    extras onto same-engine InstDrain carriers placed just before."""
    n = 0
    for bb in nc.main_func.blocks:
        insts = bb.instructions
        out = []
        changed = False
        for ins in insts:
            si = ins.sync_info
            if si is not None and len(si.on_wait) > max_waits:
                waits = list(si.on_wait)
                extra, keep = waits[:-max_waits], waits[-max_waits:]
                for i in range(0, len(extra), max_waits):
                    n += 1
                    out.append(mybir.InstNoOp(
                        name=f"wsplit-{n}",
                        sync_info=mybir.SyncInfo(
                            on_wait=extra[i:i + max_waits], on_update=[]),
                        bass_nofuse=True, engine=ins.engine))
                ins.sync_info = mybir.SyncInfo(
                    on_wait=keep, on_update=list(si.on_update))
                changed = True
            out.append(ins)
        if changed:
            insts[:] = out


# row permutation taking (v,c)-ordered 96-rows to the (c,v) DMA layout
PERM96 = np.array([3 * (i % GV) + (i // GV) for i in range(96)])


def _wp2_all(pW2_c):
    out = np.zeros((96, NP * 128), np.float64)
    for p in range(NP):
        out[6 * p:6 * p + 3, 128 * p:128 * p + 64] = pW2_c
        out[6 * p + 3:6 * p + 6, 128 * p + 64:128 * p + 128] = pW2_c
    return out


def _wv_pk2(Wv):
    out = np.zeros((64, 256), np.float64)
    out[:, 0:64] = Wv
    out[:, 128 + 64:256] = Wv
    return out


def _wqk_all(Wc):
    # lhsT selecting pair p's two voxels from the (v,c) x i layout
    out = np.zeros((96, NP * 128), np.float64)
    for p in range(NP):
        for w in range(2):
            out[6 * p + 3 * w:6 * p + 3 * w + 3,
                128 * p + 64 * w:128 * p + 64 * w + 64] = Wc
    return out


def _blockdiag(w, n):
    """n copies of w on the diagonal: [n*k, n*m]."""
    k, m = w.shape
    out = np.zeros((n * k, n * m), np.float32)
    for i in range(n):
        out[i * k:(i + 1) * k, i * m:(i + 1) * m] = w
    return out


def prep_weights(inp):
    """Host-side weight preparation (float64 for exact centering)."""
    g = lambda k: np.asarray(inp[k], np.float64)
    Wq, bq = g("Wq"), g("bq")
    Wk, bk = g("Wk"), g("bk")
    Wv, bv = g("Wv"), g("bv")
    pW1, pb1 = g("p_W1"), g("p_b1")
    pW2, pb2 = g("p_W2"), g("p_b2")
    pg, pb = g("p_ln_g"), g("p_ln_b")
    g1, b1ln = g("w_ln1_g"), g("w_ln1_b")
    W1, b1 = g("w_W1"), g("w_b1")
    g2, b2ln = g("w_ln2_g"), g("w_ln2_b")
    W2 = g("w_W2")

    assert np.all(pb == 0) and np.all(b1ln == 0) and np.all(b2ln == 0), \
        "kernel assumes LN beta == 0 (as produced by setup_inputs)"

    cen = lambda w: w - w.mean(axis=1, keepdims=True)
    Wq_c, bq_c = cen(Wq), bq - bq.mean()
    Wk_c, bk_c = cen(Wk), bk - bk.mean()
    pW2_c, pb2_c = cen(pW2), pb2 - pb2.mean()
    pW1_c, pb1_c = cen(pW1), pb1 - pb1.mean()
    W1_c, b1_c = cen(W1), b1 - b1.mean()

    bias0 = bq_c - bk_c + pb2_c            # per-d constant in w0_c

    t2 = lambda v: np.tile(v, 2).astype(np.float32)
    f = lambda v: v.astype(np.float32)
    return {
        "wq_all": f(_wqk_all(Wq_c)[PERM96]),     # [96, 16*128]
        "wk_alln": f(_wqk_all(-Wk_c)[PERM96]),   # [96, 16*128]
        "wp2_all": f(_wp2_all(pW2_c)[PERM96]),   # [96, 16*128]
        "wp1_96": f(_blockdiag(pW1_c, GV)[PERM96][:, PERM96]),   # [96, 96]
        "ones3_96": f(_blockdiag(np.ones((3, 3)), GV)[PERM96][:, PERM96]),
        "w1c_128": f(_blockdiag(W1_c, 2)),       # [128, 128]
        "w2g_128": f(_blockdiag(W2, 2)),         # W2 plain (g2 applied in rw1)
        "ones64_128": f(_blockdiag(np.ones((64, 64)), 2)),
        "ident128": np.eye(128, dtype=np.float32),
        "wv_pk2": f(_wv_pk2(Wv)),                # [64, 256]
        "bias0": t2(bias0),                      # [128]
        "bias0g1": t2(bias0 * g1),               # [128]
        "g1v": t2(g1),                           # [128]
        "b1c": t2(b1_c),                         # [128]
        "g2v": t2(g2),                           # [128]
        "b1cg2": t2(b1_c * g2),                  # [128]
        "pb1c": np.tile(pb1_c, GV)[PERM96].astype(np.float32),        # [96]
        "gpv": np.tile(pg, GV)[PERM96].astype(np.float32),            # [96]
        "pb1cgp": (np.tile(pb1_c * pg, GV))[PERM96].astype(np.float32),  # [96]
        "bv2": t2(bv),                           # [128]
        "g2_is_one": bool(np.all(g2 == 1.0)),
    }


MM_WEIGHTS = ("wq_all", "wk_alln", "wp2_all", "wp1_96", "ones3_96",
              "w1c_128", "w2g_128", "ones64_128", "wv_pk2")

WEIGHT_SHAPES = {
    "wq_all": [96, NP * 128], "wk_alln": [96, NP * 128],
    "wp2_all": [96, NP * 128],
    "wp1_96": [96, 96], "ones3_96": [96, 96],
    "w1c_128": [128, 128], "w2g_128": [128, 128], "ones64_128": [128, 128],
    "ident128": [128, 128], "wv_pk2": [64, 256],
    "bias0": [128], "bias0g1": [128], "g1v": [128], "b1c": [128],
    "g2v": [128], "b1cg2": [128],
    "pb1c": [96], "gpv": [96], "pb1cgp": [96], "bv2": [128],
}


def build_module(nv=VB, g2_is_one=True, split_waits=True, reps=1, no_io=False):
    """Build the per-core Bass module for nv voxels (nv % 32 == 0)."""
    ng = nv // GV
    nc = bass.Bass()
    xyz_d = nc.declare_dram_parameter("xyz", [nv * M * 3], f32r, isOutput=False)
    feat_d = nc.declare_dram_parameter("feat", [nv * M * D], f32, isOutput=False)
    wt = {k: nc.declare_dram_parameter(k, s, f32r if k in MM_WEIGHTS else f32,
                                       isOutput=False)
          for k, s in WEIGHT_SHAPES.items()}
    out_d = nc.declare_dram_parameter("out", [nv * M * 2 * D], f32, isOutput=True)

    def vec_tile(pool, name, p):
        t = pool.tile([p, 1], f32, tag=name)
        nc.sync.dma_start(out=t, in_=wt[name][:].unsqueeze(1))
        return t

    with tile.TileContext(nc) as tc:
        with ExitStack() as ctx:
            wpool = ctx.enter_context(tc.tile_pool(name="wts", bufs=1))
            gpool = ctx.enter_context(tc.tile_pool(name="grp", bufs=2))
            ppool = ctx.enter_context(tc.tile_pool(name="pair", bufs=2))
            spool = ctx.enter_context(tc.tile_pool(name="small", bufs=3))
            ps_w0 = ctx.enter_context(tc.tile_pool(name="psw0", bufs=2, space="PSUM"))
            ps_s = ctx.enter_context(tc.tile_pool(name="pss", bufs=2, space="PSUM"))
            ps_p = ctx.enter_context(tc.tile_pool(name="psp", bufs=2, space="PSUM"))
            ps_t = ctx.enter_context(tc.tile_pool(name="pst", bufs=1, space="PSUM"))

            # ---- preload weights ----
            w_mm = {}
            for name in ("wq_all", "wk_alln", "wp2_all", "wp1_96", "ones3_96",
                         "w1c_128", "w2g_128", "ones64_128", "ident128",
                         "wv_pk2"):
                p, q = WEIGHT_SHAPES[name]
                t = wpool.tile([p, q], f32r if name in MM_WEIGHTS else f32,
                               tag=name)
                nc.sync.dma_start(out=t, in_=wt[name][:])
                w_mm[name] = t
            vt = {}
            for name in ("bias0", "bias0g1", "g1v", "b1c", "g2v", "b1cg2"):
                vt[name] = vec_tile(wpool, name, 128)
            for name in ("pb1c", "gpv", "pb1cgp"):
                vt[name] = vec_tile(wpool, name, 96)
            vt["bv2"] = vec_tile(wpool, "bv2", 128)
            eps128 = wpool.tile([128, 1], f32, tag="eps128")
            nc.vector.memset(eps128, EPS)
            eps96 = wpool.tile([96, 1], f32, tag="eps96")
            nc.vector.memset(eps96, EPS)
            zero1 = wpool.tile([128, 1], f32, tag="zero1")
            nc.vector.memset(zero1, 0.0)

            # ---- features passthrough: out[:, :, 64:128] = feat ----
            if not no_io:
                src = bass.AP(tensor=feat_d, offset=0,
                              ap=[[M * D, nv], [D, M], [1, D]])
                dst = bass.AP(tensor=out_d, offset=D,
                              ap=[[M * 2 * D, nv], [2 * D, M], [1, D]])
                nc.sync.dma_start(out=dst, in_=src)

            for _rep in range(reps):
              for g in range(ng):
                # ---- stage AB: per-group [96, *] tiles ----
                xyzT = gpool.tile([96, M], f32r, tag="xyzT")  # (v,c) x i
                for c in range(3):
                    nc.sync.dma_start(
                        out=bass.AP(tensor=xyzT.tensor,
                                    offset=xyzT.offset + c * GV * M,
                                    ap=[[M, GV], [1, M]]),
                        in_=bass.AP(tensor=xyz_d,
                                    offset=g * GV * M * 3 + c,
                                    ap=[[M * 3, GV], [3, M]]))

                # dist = x[c,j] - x[c,i]; adist = |dist|
                dist = gpool.tile([96, M * M], f32, tag="dist")
                in0 = bass.AP(tensor=xyzT.tensor, offset=xyzT.offset,
                              ap=[xyzT.ap[0], [0, M], [1, M]]).bitcast(f32)
                in1 = bass.AP(tensor=xyzT.tensor, offset=xyzT.offset,
                              ap=[xyzT.ap[0], [1, M], [0, M]]).bitcast(f32)
                dist3 = bass.AP(tensor=dist.tensor, offset=dist.offset,
                                ap=[dist.ap[0], [M, M], [1, M]])
                nc.vector.tensor_tensor(out=dist3, in0=in0, in1=in1,
                                        op=ALU.subtract)
                adist = gpool.tile([96, M * M], f32, tag="adist")
                nc.scalar.activation(out=adist.bitcast(f32r), in_=dist, func=AF.Abs)

                # p1c = blockdiag(p_W1_c) @ adist ; LN3 stats + relu
                sqB = gpool.tile([96, M * M], f32, tag="sqB")
                rp0 = gpool.tile([96, M * M], f32, tag="rp0")
                for h in range(2):
                    p1c_t = ps_s.tile([128, 512], f32, tag="S")
                    p1c = p1c_t[:96]
                    nc.tensor.matmul(out=p1c, lhsT=w_mm["wp1_96"].bitcast(f32r),
                                     rhs=adist[:, h * 512:(h + 1) * 512].bitcast(f32r),
                                     start=True, stop=True)
                    sl = slice(h * 512, (h + 1) * 512)
                    nc.scalar.activation(out=sqB[:, sl].bitcast(f32r), in_=p1c,
                                         func=AF.Square, bias=vt["pb1c"])
                    nc.scalar.activation(out=rp0[:, sl], in_=p1c, func=AF.Relu,
                                         scale=vt["gpv"], bias=vt["pb1cgp"])
                spB = gpool.tile([96, M * M], f32, tag="spB")
                for h in range(2):
                    Sp_t = ps_s.tile([128, 512], f32, tag="S")
                    Sp = Sp_t[:96]
                    nc.tensor.matmul(out=Sp, lhsT=w_mm["ones3_96"].bitcast(f32r),
                                     rhs=sqB[:, h * 512:(h + 1) * 512].bitcast(f32r),
                                     start=True, stop=True)
                    nc.scalar.activation(out=spB[:, h * 512:(h + 1) * 512],
                                         in_=Sp, func=AF.Sqrt,
                                         scale=1.0 / 3.0, bias=eps96)
                spBinv = gpool.tile([96, M * M], f32, tag="spBinv")
                nc.vector.reciprocal(spBinv, spB)
                relu_p0 = gpool.tile([96, M * M], f32, tag="relu_p0")
                nc.vector.tensor_tensor(out=relu_p0, in0=rp0, in1=spBinv,
                                        op=ALU.mult)
                relu_p = gpool.tile([96, M * M], f32, tag="relu_p")
                nc.gpsimd.tensor_scalar(out=relu_p.bitcast(f32r), in0=relu_p0,
                                        scalar1=0.0, scalar2=None, op0=ALU.add)

                # featT: [64, (v,i)] via PE transposes; v = Wv^T @ featT + bv
                featT = gpool.tile([64, GV * M], f32, tag="featT")
                for t in range(GV * M // 128):
                    ft = spool.tile([128, D], f32, tag="ft")
                    nc.sync.dma_start(
                        out=ft,
                        in_=bass.AP(tensor=feat_d,
                                    offset=(g * GV * M + t * 128) * D,
                                    ap=[[D, 128], [1, D]]))
                    ftp = ps_t.tile([64, 128], f32, tag="tr64")
                    nc.tensor.transpose(out=ftp, in_=ft,
                                        identity=w_mm["ident128"])
                    nc.vector.tensor_copy(featT[:, t * 128:(t + 1) * 128].bitcast(f32r), ftp)
                v_sb = gpool.tile([128, NP * M], f32, tag="v_sb")
                v_ps = ps_s.tile([128, 512], f32, tag="S")
                for w in range(2):
                    v_rhs = bass.AP(tensor=featT.tensor,
                                    offset=featT.offset + w * M,
                                    ap=[featT.ap[0], [2 * M, NP], [1, M]])
                    nc.tensor.matmul(
                        out=v_ps,
                        lhsT=w_mm["wv_pk2"][:, 128 * w:128 * (w + 1)],
                        rhs=v_rhs.bitcast(f32r),
                        start=(w == 0), stop=(w == 1))
                nc.scalar.activation(out=v_sb, in_=v_ps, func=AF.Identity,
                                     bias=vt["bv2"])

                # ---- pair loop ----
                for p in range(NP):
                    sq0 = ppool.tile([128, M * M], f32, tag="sq0")
                    r0 = ppool.tile([128, M * M], f32, tag="r0")
                    s1 = ppool.tile([128, M * M], f32, tag="s1")
                    d1 = ppool.tile([128, M * M], f32, tag="d1")
                    sq2 = ppool.tile([128, M * M], f32, tag="sq2")
                    rw1 = ppool.tile([128, M * M], f32, tag="rw1")
                    s2 = ppool.tile([128, M * M], f32, tag="s2")
                    t2t = ppool.tile([128, M * M], f32, tag="t2t")
                    Et = ppool.tile([128, M * M], f32, tag="Et")

                    for h in range(2):
                        sl = slice(h * 512, (h + 1) * 512)
                        w0 = ps_w0.tile([128, 512], f32, tag="w0")
                        nc.tensor.matmul(
                            out=w0,
                            lhsT=w_mm["wp2_all"][:, 128 * p:128 * (p + 1)].bitcast(f32r),
                            rhs=relu_p[:, sl].bitcast(f32r),
                            start=True, stop=False)
                        q_rhs = bass.AP(tensor=xyzT.tensor,
                                        offset=xyzT.offset + 16 * h,
                                        ap=[xyzT.ap[0], [1, 16], [0, M]])
                        nc.tensor.matmul(
                            out=w0,
                            lhsT=w_mm["wq_all"][:, 128 * p:128 * (p + 1)].bitcast(f32r),
                            rhs=q_rhs.bitcast(f32r),
                            start=False, stop=False)
                        k_rhs = bass.AP(tensor=xyzT.tensor,
                                        offset=xyzT.offset,
                                        ap=[xyzT.ap[0], [0, 16], [1, M]])
                        nc.tensor.matmul(
                            out=w0,
                            lhsT=w_mm["wk_alln"][:, 128 * p:128 * (p + 1)].bitcast(f32r),
                            rhs=k_rhs.bitcast(f32r),
                            start=False, stop=True)
                        # sq0 = (w0 + bias0)^2 ; r0 = relu(w0*g1 + bias0*g1)
                        nc.scalar.activation(out=sq0[:, sl].bitcast(f32r), in_=w0,
                                             func=AF.Square, bias=vt["bias0"])
                        nc.scalar.activation(out=r0[:, sl].bitcast(f32r), in_=w0,
                                             func=AF.Relu,
                                             scale=vt["g1v"], bias=vt["bias0g1"])
                    for h in range(2):
                        sl = slice(h * 512, (h + 1) * 512)
                        S1 = ps_s.tile([128, 512], f32, tag="S")
                        nc.tensor.matmul(out=S1,
                                         lhsT=w_mm["ones64_128"].bitcast(f32r),
                                         rhs=sq0[:, sl].bitcast(f32r),
                                         start=True, stop=True)
                        nc.scalar.activation(out=s1[:, sl], in_=S1,
                                             func=AF.Sqrt,
                                             scale=1.0 / 64.0, bias=eps128)
                    s1i = ppool.tile([128, M * M], f32, tag="s1i")
                    nc.vector.reciprocal(s1i, s1)
                    for h in range(2):
                        sl = slice(h * 512, (h + 1) * 512)
                        P1 = ps_p.tile([128, 512], f32, tag="P")
                        nc.tensor.matmul(out=P1, lhsT=w_mm["w1c_128"].bitcast(f32r),
                                         rhs=r0[:, sl].bitcast(f32r),
                                         start=True, stop=True)
                        nc.vector.tensor_tensor(out=d1[:, sl], in0=P1,
                                                in1=s1i[:, sl], op=ALU.mult)
                    # sq2 = (d1 + b1c)^2 ; rw1 = relu((d1 + b1c)*g2)
                    nc.scalar.activation(out=sq2.bitcast(f32r), in_=d1,
                                         func=AF.Square, bias=vt["b1c"])
                    if g2_is_one:
                        zb = bass.AP(tensor=zero1.tensor, offset=zero1.offset,
                                     ap=[zero1.ap[0], [0, M * M]])
                        nc.vector.scalar_tensor_tensor(
                            out=rw1.bitcast(f32r), in0=d1, scalar=vt["b1c"],
                            in1=zb, op0=ALU.add, op1=ALU.max)
                    else:
                        nc.scalar.activation(out=rw1.bitcast(f32r), in_=d1,
                                             func=AF.Relu,
                                             scale=vt["g2v"], bias=vt["b1cg2"])
                    for h in range(2):
                        sl = slice(h * 512, (h + 1) * 512)
                        S2 = ps_s.tile([128, 512], f32, tag="S")
                        nc.tensor.matmul(out=S2,
                                         lhsT=w_mm["ones64_128"].bitcast(f32r),
                                         rhs=sq2[:, sl].bitcast(f32r),
                                         start=True, stop=True)
                        nc.scalar.activation(out=s2[:, sl], in_=S2,
                                             func=AF.Sqrt,
                                             scale=1.0 / 64.0, bias=eps128)
                    s2i = ppool.tile([128, M * M], f32, tag="s2i")
                    nc.vector.reciprocal(s2i, s2)
                    for h in range(2):
                        sl = slice(h * 512, (h + 1) * 512)
                        P2 = ps_p.tile([128, 512], f32, tag="P")
                        nc.tensor.matmul(out=P2, lhsT=w_mm["w2g_128"].bitcast(f32r),
                                         rhs=rw1[:, sl].bitcast(f32r),
                                         start=True, stop=True)
                        nc.vector.tensor_tensor(out=t2t[:, sl], in0=P2,
                                                in1=s2i[:, sl], op=ALU.mult)
                    nc.scalar.activation(out=Et, in_=t2t, func=AF.Exp)

                    # softmax pieces
                    Ssum = spool.tile([128, M], f32, tag="Ssum")
                    E3 = bass.AP(tensor=Et.tensor, offset=Et.offset,
                                 ap=[Et.ap[0], [M, M], [1, M]])
                    nc.vector.tensor_reduce(out=Ssum, in_=E3, axis=AX.X, op=ALU.add)
                    Sinv = spool.tile([128, M], f32, tag="Sinv")
                    nc.vector.reciprocal(Sinv, Ssum)
                    gt = spool.tile([128, M], f32, tag="gt")
                    Ediag = bass.AP(tensor=Et.tensor, offset=Et.offset,
                                    ap=[Et.ap[0], [M + 1, M]])
                    nc.vector.tensor_tensor(out=gt, in0=Ediag, in1=Sinv,
                                            op=ALU.mult)
                    res = spool.tile([128, M], f32, tag="res")
                    nc.vector.tensor_tensor(
                        out=res, in0=gt,
                        in1=v_sb[:, p * M:(p + 1) * M], op=ALU.mult)
                    rtp = ps_t.tile([32, 128], f32, tag="rtp")
                    nc.tensor.transpose(out=rtp, in_=res,
                                        identity=w_mm["ident128"])
                    rts = spool.tile([32, 128], f32, tag="rts")
                    nc.vector.tensor_copy(rts, rtp)
                    if not no_io:
                        nc.sync.dma_start(
                            out=bass.AP(tensor=out_d,
                                        offset=(g * GV + 2 * p) * M * 2 * D,
                                        ap=[[2 * D, M], [M * 2 * D, 2], [1, D]]),
                            in_=rts)
                    elif g == 0 and p == 0 and _rep == reps - 1:
                        nc.sync.dma_start(
                            out=bass.AP(tensor=out_d, offset=0,
                                        ap=[[2 * D, M], [M * 2 * D, 2], [1, D]]),
                            in_=rts)

    if split_waits:
        _split_sync_waits(nc)
    return nc


_module_cache = {}


def _get_module(nv, g2_is_one):
    key = (nv, g2_is_one)
    if key not in _module_cache:
        _module_cache[key] = build_module(nv, g2_is_one)
    return _module_cache[key]


def kernel(**inputs):
    w = prep_weights(inputs)
    g2_is_one = w.pop("g2_is_one")
    nc = _get_module(VB, g2_is_one)

    xyz = np.ascontiguousarray(np.asarray(inputs["pv_xyz"], np.float32))
    feat = np.ascontiguousarray(np.asarray(inputs["features"], np.float32))
    in_maps = []
    for c in range(N_CORES):
        m = {k: v for k, v in w.items()}
        m["xyz"] = xyz[c * VB:(c + 1) * VB].reshape(-1)
        m["feat"] = feat[c * VB:(c + 1) * VB].reshape(-1)
        in_maps.append(m)
    res = bass_utils.run_bass_kernel_spmd(nc, in_maps,
                                          core_ids=list(range(N_CORES)))
    out = np.concatenate(
        [res.results[c]["out"].reshape(VB, M, 2 * D) for c in range(N_CORES)],
        axis=0)
    return out
